# revision 1
# baseline (speedup 1.0000x reference)
"""Causal self-attention (B=2, T=2048, C=2048, NH=16) on 8 TRN2 NeuronCores.

Megatron-style tensor parallelism over heads: each core owns 2 heads.
Per core:
  phase 1: QKV projection in feature-major layout (fp32r matmuls),
           q^T/k^T ([d, tokens]) and V ([tokens, d]) spilled to DRAM.
  phase 2: causal attention per (batch, head) pair computed transposed:
           S^T[k,q] tiles = k^T_tile.T @ q^T_chunk, exp on ScalarE
           (PSUM->SBUF), 0/1 causal mask multiply on diagonal tiles,
           softmax denominator via an all-ones matmul (partition-dim sum),
           O^T[d,q] = V.T-free accumulation over k-tiles, divide by denom.
  phase 3: per-batch AllGather of y^T across cores (4MB shards), then each
           core computes its 256 output channels: out[t, o_slice].
Host side: transpose/shard inputs, concat per-core output column slices.
"""

import numpy as np

import concourse.bacc as bacc
import concourse.mybir as mybir
import concourse.tile as tile
from concourse.bass_utils import run_bass_kernel_spmd
from concourse.hw_specs import get_activation_tables as _get_act_tables


def _act_tables_pin_exp_ln(arch):
    """Resolve Exp and Ln only via the combined natural_log_exp set.

    The default greedy set choice puts Exp in exp_and_others and Ln in
    natural_log, so a kernel alternating exp/ln pays a ~1.3us ACT table
    load per switch. Keys/order are preserved (set ids are positional).
    """
    t = _get_act_tables(arch)
    for name, fns in t.items():
        if name != "natural_log_exp_and_others":
            fns.discard(mybir.ActivationFunctionType.Exp)
            fns.discard(mybir.ActivationFunctionType.Ln)
    return t


bacc.get_activation_tables = _act_tables_pin_exp_ln

F32R = mybir.dt.float32r
F32 = mybir.dt.float32
EXP = mybir.ActivationFunctionType.Exp
LN = mybir.ActivationFunctionType.Ln

B, T, C, NH, HS = 2, 2048, 2048, 16, 128
NCORES = 8
HPC = NH // NCORES          # heads per core
BT = B * T                  # 4096 tokens total
CT = C // 128               # 16 contraction tiles
TCH = 512                   # phase-1 token chunk
NTCH = BT // TCH            # 8
Q = 512                     # phase-2 query chunk
NQC = T // Q                # 4 per (b, h)
EXPG = 2                    # k-tiles batched per exp instruction
P3CH = 256                  # phase-3 token chunk
OSL = C // NCORES           # 256 output channels per core


def build_nc(cc: bool = True):
    nc = bacc.Bacc("TRN2", target_bir_lowering=False, num_devices=NCORES)

    # inputs are host-blocked into exact SBUF tile layouts so each load is
    # 128 fat contiguous descriptors (dispatch cost is per-descriptor)
    xT = nc.dram_tensor("xT", [NTCH, 128, CT, TCH], F32R, kind="ExternalInput")
    wqkvT = nc.dram_tensor("wqkvT", [128, CT, 6 * HS], F32R, kind="ExternalInput")
    wprojT = nc.dram_tensor("wprojT", [128, CT, OSL], F32R, kind="ExternalInput")
    masks = nc.dram_tensor("masks", [128, 4, Q], F32R, kind="ExternalInput")
    ones = nc.dram_tensor("ones", [128, 128], F32R, kind="ExternalInput")
    out_loc = nc.dram_tensor("out_loc", [BT, OSL], F32, kind="ExternalOutput")

    # per-batch spill tensors so batch-0 attention loads don't wait on
    # batch-1 projection writes
    q_dram = [nc.dram_tensor(f"q_dram{b}", [HPC * HS, T], F32R) for b in range(B)]
    k_dram = [nc.dram_tensor(f"k_dram{b}", [HPC * HS, T], F32R) for b in range(B)]
    # V spilled in [p, k-tile, d] blocks so the phase-2 reload is contiguous
    v_dram = [
        [nc.dram_tensor(f"v_dram{b}_{hl}", [128, CT, HS], F32R) for hl in range(HPC)]
        for b in range(B)
    ]
    # per-(batch, head) gather shards: earlier collectives, more overlap.
    # yg[b][hl] rows are head-major with head = 2*core + hl; the host
    # permutes wprojT rows to match (even heads then odd heads).
    y_loc = [[nc.dram_tensor(f"y_loc{b}_{hl}", [HS, T], F32R) for hl in range(HPC)] for b in range(B)]
    yg = [
        [
            nc.dram_tensor(
                f"yg{b}_{hl}", [NCORES * HS, T], F32R,
                addr_space="Shared" if cc else "Local",
            )
            for hl in range(HPC)
        ]
        for b in range(B)
    ]

    with tile.TileContext(nc) as tc:
        with (
            # persistent pools: disjoint from the phase-1 pools so phase-2's
            # q/k loads can prefetch while phase-1 still computes
            tc.tile_pool(name="const2", bufs=1) as const2,
            tc.tile_pool(name="qp", bufs=2) as qp,
            tc.tile_pool(name="kp", bufs=2) as kp,
        ):
            masks_sb = const2.tile([128, 4, Q], F32R)
            nc.gpsimd.dma_start(out=masks_sb, in_=masks[:])
            ones_sb = const2.tile([128, 128], F32R)
            nc.gpsimd.dma_start(out=ones_sb, in_=ones[:])

            # ---------------- phase 1: QKV projection ----------------
            with (
                tc.tile_pool(name="wq", bufs=1) as wq_pool,
                tc.tile_pool(name="xin", bufs=3) as xin,
                tc.tile_pool(name="qkst", bufs=4) as qkst,
                tc.tile_pool(name="vst", bufs=4) as vst,
                tc.tile_pool(name="ps1", bufs=4, space="PSUM") as ps1,
                tc.tile_pool(name="psv", bufs=4, space="PSUM") as psv,
            ):
                wq_sb = wq_pool.tile([128, CT, 6 * HS], F32R)
                x_first = xin.tile([128, CT, TCH], F32R, name="x_sb")
                # interleave w/x row-group loads so the first matmuls (which
                # need only row-group 0 of each) start as early as possible
                for g in range(4):
                    nc.sync.dma_start(
                        out=wq_sb[:, 4 * g : 4 * g + 4, :],
                        in_=wqkvT[:, 4 * g : 4 * g + 4, :],
                    )
                    nc.sync.dma_start(
                        out=x_first[:, 4 * g : 4 * g + 4, :],
                        in_=xT[0, :, 4 * g : 4 * g + 4, :],
                    )
                for tch in range(NTCH):
                    bb, tin = tch // (NTCH // B), (tch % (NTCH // B)) * TCH
                    tsl = slice(tin, tin + TCH)
                    if tch == 0:
                        x_sb = x_first
                    else:
                        x_sb = xin.tile([128, CT, TCH], F32R, name="x_sb")
                        for g in range(4):
                            nc.sync.dma_start(
                                out=x_sb[:, 4 * g : 4 * g + 4, :],
                                in_=xT[tch, :, 4 * g : 4 * g + 4, :],
                            )
                    for ot in range(4):  # q_h0, q_h1, k_h0, k_h1
                        pq = ps1.tile([128, TCH], F32)
                        for ci in range(CT):
                            nc.tensor.matmul(
                                pq[:],
                                wq_sb[:, ci, ot * 128 : (ot + 1) * 128],
                                x_sb[:, ci, :],
                                start=(ci == 0),
                                stop=(ci == CT - 1),
                            )
                        st = qkst.tile([128, TCH], F32R)
                        nc.vector.tensor_copy(out=st[:], in_=pq[:])
                        dst = (q_dram if ot < 2 else k_dram)[bb]
                        hl = ot % 2
                        nc.sync.dma_start(
                            out=dst[hl * 128 : (hl + 1) * 128, tsl], in_=st[:]
                        )
                    for tt in range(TCH // 128):  # V in natural [token, d] layout
                        pv = psv.tile([128, 2 * HS], F32)
                        for ci in range(CT):
                            nc.tensor.matmul(
                                pv[:],
                                x_sb[:, ci, tt * 128 : (tt + 1) * 128],
                                wq_sb[:, ci, 4 * HS : 6 * HS],
                                start=(ci == 0),
                                stop=(ci == CT - 1),
                            )
                        sv = vst.tile([128, 2 * HS], F32R)
                        nc.vector.tensor_copy(out=sv[:], in_=pv[:])
                        ktg = (tin + tt * 128) // 128
                        for hl in range(HPC):
                            nc.sync.dma_start(
                                out=v_dram[bb][hl][:, ktg, :],
                                in_=sv[:, hl * HS : (hl + 1) * HS],
                            )

            # ---------------- phases 2+3: attention, gather, out-proj ----------------
            with (
                tc.tile_pool(name="vp", bufs=2) as vp,
                tc.tile_pool(name="esp", bufs=2) as esp,
                tc.tile_pool(name="rp", bufs=2) as rp,
                tc.tile_pool(name="yst", bufs=2) as yst,
                tc.tile_pool(name="wp", bufs=1) as wp_pool,
                tc.tile_pool(name="ygp", bufs=3) as ygp,
                tc.tile_pool(name="ost", bufs=2) as ost,
                tc.tile_pool(name="ps_s", bufs=2, space="PSUM") as ps_s,
                tc.tile_pool(name="ps_d", bufs=1, space="PSUM") as ps_d,
                tc.tile_pool(name="ps_o", bufs=2, space="PSUM") as ps_o,
                tc.tile_pool(name="ps3", bufs=1, space="PSUM") as ps3,
            ):
                wp_sb = wp_pool.tile([128, CT, OSL], F32R)
                nc.gpsimd.dma_start(out=wp_sb, in_=wprojT[:])

                # denom/AV matmuls are emitted one chunk late so the in-order
                # PE queue has S-matmuls of the next chunk to chew on while
                # the last exp group of the current chunk drains via ACT/DVE
                pending: list = []

                def flush_pending():
                    while pending:
                        pending.pop(0)()

                def denom_av(b, hl, qc, nk, es, v_sb, qsl):
                    dp = ps_d.tile([128, Q], F32, name="dp")
                    for kt in range(nk):
                        nc.tensor.matmul(
                            dp[:], ones_sb[:], es[:, kt, :],
                            start=(kt == 0), stop=(kt == nk - 1),
                        )
                    # 1/x as exp(-ln(x)) on ScalarE: DVE's reciprocal
                    # intrinsic costs ~3.4us/tile and clogs the DVE queue
                    ln_sb = rp.tile([128, Q], F32, tag="lnt", name="ln_sb")
                    nc.scalar.activation(out=ln_sb[:], in_=dp[:], func=LN)
                    r_sb = rp.tile([128, Q], F32, tag="rsb", name="r_sb")
                    nc.scalar.activation(out=r_sb[:], in_=ln_sb[:], func=EXP, scale=-1.0)
                    po = ps_o.tile([128, Q], F32, name="po")
                    for kt in range(nk):
                        nc.tensor.matmul(
                            po[:], v_sb[:, kt, :], es[:, kt, :],
                            start=(kt == 0), stop=(kt == nk - 1),
                        )
                    y_sb = yst.tile([128, Q], F32R, name="y_sb")
                    nc.vector.tensor_mul(out=y_sb[:], in0=po[:], in1=r_sb[:])
                    nc.sync.dma_start(out=y_loc[b][hl][:, qsl], in_=y_sb[:])

                def attention_pair(b: int, hl: int):
                    hsl = slice(hl * 128, (hl + 1) * 128)
                    q_sb = qp.tile([128, T], F32R, name="q_sb")
                    nc.gpsimd.dma_start(out=q_sb, in_=q_dram[b][hsl, :])
                    k_sb = kp.tile([128, CT, 128], F32R, name="k_sb")
                    nc.gpsimd.dma_start(
                        out=k_sb,
                        in_=k_dram[b][hsl, :].rearrange("p (kt t) -> p kt t", t=128),
                    )
                    v_sb = vp.tile([128, CT, HS], F32R, name="v_sb")
                    nc.sync.dma_start(out=v_sb, in_=v_dram[b][hl][:])
                    for qc in range(NQC):
                        nk = (qc + 1) * (Q // 128)  # causal: k-tiles 0..nk-1
                        qsl = slice(qc * Q, (qc + 1) * Q)
                        es = esp.tile([128, CT, Q], F32R, name="es")
                        for g in range(nk // EXPG):
                            sp = ps_s.tile([128, EXPG * Q], F32, name="sp")
                            for j in range(EXPG):
                                kt = g * EXPG + j
                                nc.tensor.matmul(
                                    sp[:, j * Q : (j + 1) * Q],
                                    k_sb[:, kt, :],
                                    q_sb[:, qsl],
                                    start=True,
                                    stop=True,
                                )
                            nc.scalar.activation(
                                out=es[:, g * EXPG : (g + 1) * EXPG, :].rearrange(
                                    "p a q -> p (a q)"
                                ),
                                in_=sp[:],
                                func=EXP,
                            )
                            if g * EXPG >= nk - 4:  # diagonal groups -> 0/1 mask
                                a0 = g * EXPG - (nk - 4)
                                nc.vector.tensor_tensor(
                                    es[:, g * EXPG : (g + 1) * EXPG, :],
                                    es[:, g * EXPG : (g + 1) * EXPG, :],
                                    masks_sb[:, a0 : a0 + EXPG, :],
                                    mybir.AluOpType.mult,
                                )
                        flush_pending()
                        pending.append(
                            lambda b=b, hl=hl, qc=qc, nk=nk, es=es, v_sb=v_sb, qsl=qsl: denom_av(
                                b, hl, qc, nk, es, v_sb, qsl
                            )
                        )

                def gather(b: int, hl: int):
                    if cc:
                        nc.gpsimd.collective_compute(
                            "AllGather",
                            mybir.AluOpType.bypass,
                            replica_groups=[list(range(NCORES))],
                            ins=[y_loc[b][hl].ap()],
                            outs=[yg[b][hl].ap()],
                        )
                    else:  # timing-only variant: no inter-core traffic
                        nc.sync.dma_start(out=yg[b][hl][:HS, :], in_=y_loc[b][hl].ap())

                def out_proj(b: int):
                    # contract over even-head gather rows then odd-head rows;
                    # wprojT rows are host-permuted to match
                    for ch in range(T // P3CH):
                        csl = slice(ch * P3CH, (ch + 1) * P3CH)
                        yg_sb = ygp.tile([128, CT, P3CH], F32R, name="yg_sb")
                        for hl in range(HPC):
                            # split dispatch across two DMA paths
                            eng = nc.gpsimd if hl == 0 else nc.sync
                            eng.dma_start(
                                out=yg_sb[:, hl * (CT // 2) : (hl + 1) * (CT // 2), :],
                                in_=yg[b][hl][:, csl].rearrange(
                                    "(ko p) t -> p ko t", p=128
                                ),
                            )
                        for tt in range(P3CH // 128):
                            po = ps3.tile([128, OSL], F32, name="po3")
                            for ci in range(CT):
                                nc.tensor.matmul(
                                    po[:],
                                    yg_sb[:, ci, tt * 128 : (tt + 1) * 128],
                                    wp_sb[:, ci, :],
                                    start=(ci == 0),
                                    stop=(ci == CT - 1),
                                )
                            o_sb = ost.tile([128, OSL], F32, name="o_sb")
                            nc.vector.tensor_copy(out=o_sb[:], in_=po[:])
                            nc.sync.dma_start(
                                out=out_loc[
                                    b * T + ch * P3CH + tt * 128 : b * T
                                    + ch * P3CH
                                    + (tt + 1) * 128,
                                    :,
                                ],
                                in_=o_sb[:],
                            )

                attention_pair(0, 0)
                attention_pair(0, 1)
                flush_pending()
                gather(0, 0)
                gather(0, 1)
                attention_pair(1, 0)
                out_proj(0)
                attention_pair(1, 1)
                flush_pending()
                gather(1, 0)
                gather(1, 1)
                out_proj(1)

    nc.finalize()
    return nc


def prep_inputs(x: np.ndarray, w_attn: np.ndarray, w_proj: np.ndarray):
    """Host-side sharding/layout. Returns per-core input maps."""
    # blocked to [chunk, partition, c-tile, token]
    xT = np.ascontiguousarray(
        x.reshape(NTCH, TCH, CT, 128).transpose(0, 3, 2, 1)
    )
    wq, wk, wv = w_attn[:C], w_attn[C : 2 * C], w_attn[2 * C :]
    scale = np.float32(1.0 / np.sqrt(HS))
    kk = np.arange(128, dtype=np.int64)[:, None, None]
    aa = np.arange(4, dtype=np.int64)[None, :, None]
    qq = np.arange(Q, dtype=np.int64)[None, None, :]
    masks = (128 * aa + kk <= qq).astype(np.float32)
    in_maps = []
    for c in range(NCORES):
        h0 = HPC * c
        rows = slice(h0 * HS, (h0 + HPC) * HS)
        wqkvT = np.ascontiguousarray(
            np.concatenate([wq[rows] * scale, wk[rows], wv[rows]], axis=0)
            .T.reshape(CT, 128, 6 * HS)
            .transpose(1, 0, 2)
        )
        # rows permuted to the per-(batch,head) gather layout: the gathers
        # concatenate cores, so channel order is even heads (hl=0) then odd
        # heads (hl=1), head = 2*core + hl
        perm = np.concatenate(
            [
                np.arange(HS) + h * HS
                for hl in range(HPC)
                for h in range(hl, NH, HPC)
            ]
        )
        wprojT = np.ascontiguousarray(
            w_proj[c * OSL : (c + 1) * OSL, perm]
            .T.reshape(CT, 128, OSL)
            .transpose(1, 0, 2)
        )
        in_maps.append(
            {
                "xT": xT,
                "wqkvT": wqkvT,
                "wprojT": wprojT,
                "masks": masks,
                "ones": np.ones((128, 128), dtype=np.float32),
            }
        )
    return in_maps


_CACHE: dict = {}


def _get_nc(cc: bool = True):
    key = ("nc", cc)
    if key not in _CACHE:
        _CACHE[key] = build_nc(cc=cc)
    return _CACHE[key]


def run(x, w_attn, w_proj, cc: bool = True, **spmd_kwargs):
    nc = _get_nc(cc=cc)
    in_maps = prep_inputs(
        np.asarray(x, dtype=np.float32),
        np.asarray(w_attn, dtype=np.float32),
        np.asarray(w_proj, dtype=np.float32),
    )
    res = run_bass_kernel_spmd(nc, in_maps, list(range(NCORES)), **spmd_kwargs)
    out = np.concatenate([res.results[c]["out_loc"] for c in range(NCORES)], axis=1)
    return out.reshape(B, T, C), res


def kernel(x, w_attn, w_proj):
    out, _ = run(x, w_attn, w_proj, cc=True)
    return out



# revision 12
# speedup vs baseline: 1.2715x; 1.2715x over previous
"""Causal self-attention (B=2, T=2048, C=2048, NH=16) on 8 TRN2 NeuronCores.

Megatron-style tensor parallelism over heads: each core owns 2 heads.
All matmul operands are bf16 (PE rate equals fp32r at these shapes, but
DMA/SBUF halve); PSUM accumulation stays fp32.

Per core, fully fused single pass over 8 token chunks of 512:
  - QKV projection chunk-by-chunk, q/k/v kept SBUF-resident (no spills).
  - Attention interleaved per 512-query chunk right behind the QKV chunk
    that completes its causal k-prefix: S^T tiles = k_tile.T @ q_chunk,
    exp on ScalarE (PSUM->SBUF, bf16 out), 0/1 mask multiply on diagonal
    tiles, denominator via all-ones matmul, O^T = V-stationary
    accumulation, divide by denominator on DVE.
  - Output projection per chunk from SBUF-resident y (contract the 256
    local head channels against w_proj columns), partial [512, 2048]
    written to DRAM, then a per-chunk ReduceScatter(add) across the 8
    cores produces each core's final 64-token slice.
Denominator/AV/out-proj matmuls are emitted as small FIFO fragments
interleaved between later S-matmul groups so the in-order PE queue never
head-of-line blocks on the exp pipeline. All PSUM tiles used by deferred
fragments are allocated inside the fragment (emission order == pool
rotation order).
Host side: cast/shard inputs to bf16, reassemble the scattered output.
"""

import numpy as np
import ml_dtypes

import concourse.bacc as bacc
import concourse.mybir as mybir
import concourse.tile as tile
from concourse.bass_utils import run_bass_kernel_spmd
from concourse.hw_specs import get_activation_tables as _get_act_tables


def _act_tables_pin_exp_ln(arch):
    """Resolve Exp and Ln only via the combined natural_log_exp set so the
    kernel never pays an ACT table reload when alternating exp/ln."""
    t = _get_act_tables(arch)
    for name, fns in t.items():
        if name != "natural_log_exp_and_others":
            fns.discard(mybir.ActivationFunctionType.Exp)
            fns.discard(mybir.ActivationFunctionType.Ln)
    return t


bacc.get_activation_tables = _act_tables_pin_exp_ln

BF16 = mybir.dt.bfloat16
F32 = mybir.dt.float32
EXP = mybir.ActivationFunctionType.Exp
LN = mybir.ActivationFunctionType.Ln

B, T, C, NH, HS = 2, 2048, 2048, 16, 128
NCORES = 8
HPC = NH // NCORES          # heads per core
BT = B * T                  # 4096 tokens total
CT = C // 128               # 16 contraction tiles
TCH = 512                   # token chunk (both projection and query chunk)
NCH = BT // TCH             # 8 chunks
NQC = T // TCH              # 4 query chunks per batch
EXPG = 2                    # k-tiles batched per exp instruction
NOS = 4                     # out-proj output-column slices
OSS = C // NOS              # 512


def build_nc(cc: bool = True):
    nc = bacc.Bacc("TRN2", target_bir_lowering=False, num_devices=NCORES)

    # host-blocked so every load is 128 fat contiguous descriptors
    xT = nc.dram_tensor("xT", [NCH, 128, CT, TCH], BF16, kind="ExternalInput")
    wqkvT = nc.dram_tensor("wqkvT", [128, CT, 6 * HS], BF16, kind="ExternalInput")
    wpT = nc.dram_tensor("wpT", [128, HPC, C], BF16, kind="ExternalInput")
    masks = nc.dram_tensor("masks", [128, 4, TCH], BF16, kind="ExternalInput")
    ones = nc.dram_tensor("ones", [128, 128], BF16, kind="ExternalInput")
    # per-chunk partial output (full 2048 channels) and its reduce-scatter
    pout = [nc.dram_tensor(f"pout{ch}", [TCH, C], BF16) for ch in range(NCH)]
    rs_buf = [
        nc.dram_tensor(f"rs_buf{ch}", [TCH * C // NCORES], BF16)
        for ch in range(NCH)
    ]
    rs_out = nc.dram_tensor(
        "rs_out", [NCH, TCH * C // NCORES], BF16, kind="ExternalOutput"
    )

    with tile.TileContext(nc) as tc:
        with (
            tc.tile_pool(name="const", bufs=1) as const,
            tc.tile_pool(name="wq", bufs=1) as wq_pool,
            tc.tile_pool(name="wp", bufs=1) as wp_pool,
            tc.tile_pool(name="xin", bufs=3) as xin,
            tc.tile_pool(name="qp", bufs=2) as qp,
            tc.tile_pool(name="kp", bufs=2) as kp,
            tc.tile_pool(name="vp", bufs=2) as vp,
            tc.tile_pool(name="esp", bufs=2) as esp,
            tc.tile_pool(name="yp", bufs=2) as yp,
            tc.tile_pool(name="rp", bufs=2) as rp,
            tc.tile_pool(name="op", bufs=3) as op_pool,
            tc.tile_pool(name="ps_s", bufs=2, space="PSUM") as ps_s,
            tc.tile_pool(name="ps_dp", bufs=1, space="PSUM") as ps_dp,
            tc.tile_pool(name="ps_po", bufs=1, space="PSUM") as ps_po,
            tc.tile_pool(name="ps_pb", bufs=2, space="PSUM") as ps_pb,
        ):
            wq_sb = wq_pool.tile([128, CT, 6 * HS], BF16)

            # startup: interleave w (gpsimd queue) and x chunk 0 (sync queue)
            x_tiles: dict = {}
            x_first = xin.tile([128, CT, TCH], BF16, name="x_sb")
            x_tiles[0] = x_first
            for g in range(4):
                nc.gpsimd.dma_start(
                    out=wq_sb[:, 4 * g : 4 * g + 4, :],
                    in_=wqkvT[:, 4 * g : 4 * g + 4, :],
                )
                nc.sync.dma_start(
                    out=x_first[:, 4 * g : 4 * g + 4, :],
                    in_=xT[0, :, 4 * g : 4 * g + 4, :],
                )
            masks_sb = const.tile([128, 4, TCH], BF16)
            nc.gpsimd.dma_start(out=masks_sb, in_=masks[:])
            ones_sb = const.tile([128, 128], BF16)
            nc.gpsimd.dma_start(out=ones_sb, in_=ones[:])
            wp_sb = wp_pool.tile([128, HPC, C], BF16)
            nc.gpsimd.dma_start(out=wp_sb, in_=wpT[:])

            # qkv SBUF residency: one tile per batch, rotating bufs=2
            q_sb: dict = {}
            k_sb: dict = {}
            v_sb: dict = {}

            # deferred small PE fragments (denominator / AV / out-proj)
            # popped FIFO between S-groups so the PE never runs dry
            pending: list = []

            def pop_pending(n):
                for _ in range(min(n, len(pending))):
                    pending.pop(0)()

            def flush_pending():
                while pending:
                    pending.pop(0)()

            def load_x(tch):
                x_t = xin.tile([128, CT, TCH], BF16, name="x_sb")
                x_tiles[tch] = x_t
                for g in range(4):
                    nc.sync.dma_start(
                        out=x_t[:, 4 * g : 4 * g + 4, :],
                        in_=xT[tch, :, 4 * g : 4 * g + 4, :],
                    )

            def qkv_chunk(tch):
                bb, tin = tch // NQC, (tch % NQC) * TCH
                tsl = slice(tin, tin + TCH)
                if bb not in q_sb:
                    q_sb[bb] = qp.tile([128, HPC, T], BF16, name="q_sb")
                    k_sb[bb] = kp.tile([128, HPC, T], BF16, name="k_sb")
                    v_sb[bb] = vp.tile([128, CT, HPC * HS], BF16, name="v_sb")
                x_t = x_tiles.pop(tch)
                for ot in range(4):  # q_h0, q_h1, k_h0, k_h1
                    pq = ps_s.tile([128, EXPG, TCH], F32, name="sp")
                    for ci in range(CT):
                        nc.tensor.matmul(
                            pq[:, 0, :],
                            wq_sb[:, ci, ot * 128 : (ot + 1) * 128],
                            x_t[:, ci, :],
                            start=(ci == 0),
                            stop=(ci == CT - 1),
                        )
                    dst = (q_sb if ot < 2 else k_sb)[bb]
                    nc.vector.tensor_copy(out=dst[:, ot % 2, tsl], in_=pq[:, 0, :])
                    pop_pending(2)
                for tt in range(TCH // 128):  # V in [token, d] layout
                    pv = ps_pb.tile([128, TCH], F32, name="pb")
                    for ci in range(CT):
                        nc.tensor.matmul(
                            pv[:, : HPC * HS],
                            x_t[:, ci, tt * 128 : (tt + 1) * 128],
                            wq_sb[:, ci, 4 * HS : 6 * HS],
                            start=(ci == 0),
                            stop=(ci == CT - 1),
                        )
                    ktg = (tch % NQC) * 4 + tt
                    nc.vector.tensor_copy(
                        out=v_sb[bb][:, ktg, :], in_=pv[:, : HPC * HS]
                    )
                    pop_pending(2)

            def denom_av(b, hl, nk, es, y_t):
                """Queue denominator + AV + divide for one (chunk, head) as
                small PE fragments. PSUM tiles allocated at pop time."""
                dp_box: list = []
                po_box: list = []
                r_box: list = []

                def dp_frag(k0, k1):
                    if not dp_box:
                        dp_box.append(ps_dp.tile([128, TCH], F32, name="dp"))
                    dp = dp_box[0]
                    for kt in range(k0, k1):
                        nc.tensor.matmul(
                            dp[:], ones_sb[:], es[:, kt, :],
                            start=(kt == 0), stop=(kt == nk - 1),
                            skip_group_check=True,
                        )

                def recip():
                    # 1/x as exp(-ln(x)) on ScalarE (DVE reciprocal is slow)
                    ln_t = rp.tile([128, TCH], F32, tag="lnt", name="ln_sb")
                    nc.scalar.activation(out=ln_t[:], in_=dp_box[0][:], func=LN)
                    r_t = rp.tile([128, TCH], F32, tag="rsb", name="r_sb")
                    nc.scalar.activation(out=r_t[:], in_=ln_t[:], func=EXP, scale=-1.0)
                    r_box.append(r_t)

                def po_frag(k0, k1):
                    if not po_box:
                        po_box.append(ps_po.tile([128, TCH], F32, name="po"))
                    po = po_box[0]
                    for kt in range(k0, k1):
                        nc.tensor.matmul(
                            po[:], v_sb[b][:, kt, hl * HS : (hl + 1) * HS],
                            es[:, kt, :],
                            start=(kt == 0), stop=(kt == nk - 1),
                            skip_group_check=True,
                        )

                def div():
                    nc.vector.tensor_mul(
                        out=y_t[:, hl, :], in0=po_box[0][:], in1=r_box[0][:]
                    )

                for k0 in range(0, nk, 4):
                    pending.append(lambda k0=k0: dp_frag(k0, min(k0 + 4, nk)))
                pending.append(recip)
                for k0 in range(0, nk, 4):
                    pending.append(lambda k0=k0: po_frag(k0, min(k0 + 4, nk)))
                pending.append(div)

            def out_proj(ch, y_t):
                """Queue the chunk's out-projection as per-(tt,os) fragments."""
                o_tiles: dict = {}

                def frag(tt, osl):
                    if osl == 0:
                        o_tiles[tt] = op_pool.tile([128, C], BF16, name="o_sb")
                    po3 = ps_pb.tile([128, TCH], F32, name="pb")
                    for hl in range(HPC):
                        nc.tensor.matmul(
                            po3[:],
                            y_t[:, hl, tt * 128 : (tt + 1) * 128],
                            wp_sb[:, hl, osl * OSS : (osl + 1) * OSS],
                            start=(hl == 0),
                            stop=(hl == HPC - 1),
                        )
                    dst = o_tiles[tt][:, osl * OSS : (osl + 1) * OSS]
                    if osl < 2:
                        nc.vector.tensor_copy(out=dst, in_=po3[:])
                    else:
                        nc.scalar.copy(out=dst, in_=po3[:])
                    if osl == NOS - 1:
                        nc.sync.dma_start(
                            out=pout[ch][tt * 128 : (tt + 1) * 128, :],
                            in_=o_tiles[tt],
                        )

                for tt in range(TCH // 128):
                    for osl in range(NOS):
                        pending.append(lambda tt=tt, osl=osl: frag(tt, osl))

                def rs():
                    if cc:
                        nc.gpsimd.collective_compute(
                            "ReduceScatter",
                            mybir.AluOpType.add,
                            replica_groups=[list(range(NCORES))],
                            ins=[pout[ch].ap()],
                            outs=[rs_buf[ch].ap()],
                        )
                        nc.gpsimd.dma_start(
                            out=rs_out[ch], in_=rs_buf[ch].ap()
                        )
                    else:  # timing-only variant: no inter-core traffic
                        nc.sync.dma_start(
                            out=rs_out[ch].rearrange("(a b) -> a b", b=C),
                            in_=pout[ch][: TCH // NCORES, :],
                        )

                pending.append(rs)

            def attn_chunk(b, qc):
                nk = (qc + 1) * (TCH // 128)  # causal: k-tiles 0..nk-1
                qsl = slice(qc * TCH, (qc + 1) * TCH)
                y_t = yp.tile([128, HPC, TCH], BF16, name="y_sb")
                for hl in range(HPC):
                    es = esp.tile([128, CT, TCH], BF16, name="es")
                    for g in range(nk // EXPG):
                        sp = ps_s.tile([128, EXPG, TCH], F32, name="sp")
                        for j in range(EXPG):
                            kt = g * EXPG + j
                            nc.tensor.matmul(
                                sp[:, j, :],
                                k_sb[b][:, hl, kt * 128 : (kt + 1) * 128],
                                q_sb[b][:, hl, qsl],
                                start=True,
                                stop=True,
                            )
                        nc.scalar.activation(
                            out=es[:, g * EXPG : (g + 1) * EXPG, :].rearrange(
                                "p a q -> p (a q)"
                            ),
                            in_=sp[:].rearrange("p a q -> p (a q)"),
                            func=EXP,
                        )
                        if g * EXPG >= nk - 4:  # diagonal groups -> 0/1 mask
                            a0 = g * EXPG - (nk - 4)
                            nc.vector.tensor_tensor(
                                es[:, g * EXPG : (g + 1) * EXPG, :],
                                es[:, g * EXPG : (g + 1) * EXPG, :],
                                masks_sb[:, a0 : a0 + EXPG, :],
                                mybir.AluOpType.mult,
                            )
                        pop_pending(2)
                    denom_av(b, hl, nk, es, y_t)
                out_proj(b * NQC + qc, y_t)

            # ---------------- schedule ----------------
            load_x(1)
            for tch in range(NCH):
                if tch + 2 < NCH:
                    load_x(tch + 2)
                qkv_chunk(tch)
                attn_chunk(tch // NQC, tch % NQC)
            flush_pending()

    nc.finalize()
    return nc


def prep_inputs(x: np.ndarray, w_attn: np.ndarray, w_proj: np.ndarray):
    """Host-side sharding/layout. Returns per-core input maps."""
    bf = ml_dtypes.bfloat16
    xT = np.ascontiguousarray(
        x.reshape(NCH, TCH, CT, 128).transpose(0, 3, 2, 1)
    ).astype(bf)
    wq, wk, wv = w_attn[:C], w_attn[C : 2 * C], w_attn[2 * C :]
    scale = np.float32(1.0 / np.sqrt(HS))
    kk = np.arange(128, dtype=np.int64)[:, None, None]
    aa = np.arange(4, dtype=np.int64)[None, :, None]
    qq = np.arange(TCH, dtype=np.int64)[None, None, :]
    masks = (128 * aa + kk <= qq).astype(bf)
    ones = np.ones((128, 128), dtype=bf)
    in_maps = []
    for c in range(NCORES):
        h0 = HPC * c
        rows = slice(h0 * HS, (h0 + HPC) * HS)
        wqkvT = np.ascontiguousarray(
            np.concatenate([wq[rows] * scale, wk[rows], wv[rows]], axis=0)
            .T.reshape(CT, 128, 6 * HS)
            .transpose(1, 0, 2)
        ).astype(bf)
        # wpT[c]: rows = this core's 256 y channels, all 2048 out channels
        wpT = np.ascontiguousarray(
            w_proj[:, c * HPC * HS : (c + 1) * HPC * HS]
            .T.reshape(HPC, 128, C)
            .transpose(1, 0, 2)
        ).astype(bf)
        in_maps.append(
            {"xT": xT, "wqkvT": wqkvT, "wpT": wpT, "masks": masks, "ones": ones}
        )
    return in_maps


_CACHE: dict = {}


def _get_nc(cc: bool = True):
    key = ("nc", cc)
    if key not in _CACHE:
        _CACHE[key] = build_nc(cc=cc)
    return _CACHE[key]


def run(x, w_attn, w_proj, cc: bool = True, **spmd_kwargs):
    nc = _get_nc(cc=cc)
    in_maps = prep_inputs(
        np.asarray(x, dtype=np.float32),
        np.asarray(w_attn, dtype=np.float32),
        np.asarray(w_proj, dtype=np.float32),
    )
    res = run_bass_kernel_spmd(nc, in_maps, list(range(NCORES)), **spmd_kwargs)
    # rs_out[c][ch] holds tokens [64c .. 64c+64) of chunk ch, all 2048 chans
    out = np.zeros((BT, C), dtype=np.float32)
    tpc = TCH // NCORES  # 64 tokens per core per chunk
    for c in range(NCORES):
        r = np.asarray(res.results[c]["rs_out"], dtype=np.float32).reshape(
            NCH, tpc, C
        )
        for ch in range(NCH):
            t0 = ch * TCH + c * tpc
            out[t0 : t0 + tpc, :] = r[ch]
    return out.reshape(B, T, C), res


def kernel(x, w_attn, w_proj):
    out, _ = run(x, w_attn, w_proj, cc=True)
    return out
